# revision 29
# baseline (speedup 1.0000x reference)
"""Distributed Trainium2 Bass kernel for the contextual-attention module.

Sharding (per hint): data-parallel over batch (2 samples x 4 cores); within a
sample the L=4096 kernel axis is sharded 4 ways (1024 kernels / 16 patch-center
rows per core).  Per core, the 64x64 spatial grid is processed in 8 chunks of
8 rows (CS=512 positions).

fp8 DoubleRow geometry: the box-filtered feature map and the attn maps are
stored "spaced" -- one fp8 value every 2 bytes, rows 132 B apart -- so every
3x3-shift window becomes a flat stride-2 byte stream with an even start
offset, which satisfies the dual-fp8 ISA rules (rhs free-AP depth <= 2, 2B
start alignment).  An out-of-range x shift wraps into the neighbouring
zeroed slot, so no halo copies are needed.

  GEMM1  scores[l, s] = sum_{c,d} kern[l,c,d] * boxfeat[c, s+d]  (3x3
         box-sum commuted onto the feature map); per l-tile and half-chunk,
         4 DoubleRow matmuls (paired shifts) + 1 single accumulate in PSUM.
         Kernel L2 normalization rides as a per-partition activation scale
         on the PSUM->SBUF copy (rnorm[l]).
  softmax over the full L axis is flash-style: local max via one gpsimd
         partition_all_reduce, exp against it (strided write into the fp8
         attn map), local sums via fp8 ones-matmuls; (max, sum) stat rows
         for chunk pairs go through one 4-core AllGather + local combine
         with ~2 pipeline periods of latency slack.
  GEMM2  is output-pixel-major: per half-chunk, 9 shifted windows x 4
         l-tile-pairs of DoubleRow matmuls (kernel index flipped, rnorm
         folded into kern_lc) accumulate into one PSUM tile -- no canvas
         overlap-add.
  blend  out = psum * (fac * (1-mask)/9) + feat*mask/4; ReduceScatter over
         channels per chunk pair (last two chunks scatter individually to
         shorten the tail).

The PE instruction stream (GEMM2(k-2), GEMM1(k+1), sums(k) per iteration)
never waits on a collective, keeping the HAM clock gate warm.
"""

import os
import sys
import types

for _p in ("/opt/trn_rl_repo",):
    if os.path.isdir(_p) and _p not in sys.path:
        sys.path.append(_p)


def _ensure_axon_hooks():
    """Make antenv.axon_hooks importable so bass_utils trace mode never
    crashes on the import (hook may still be None -> tracing is skipped)."""
    try:
        import antenv.axon_hooks  # noqa: F401
        return
    except Exception:
        pass
    try:
        import antenv
        mod = types.ModuleType("antenv.axon_hooks")
        mod._hook = None

        def set_axon_ntff_profile_hook(hook):
            mod._hook = hook

        def get_axon_ntff_profile_hook():
            return mod._hook

        mod.set_axon_ntff_profile_hook = set_axon_ntff_profile_hook
        mod.get_axon_ntff_profile_hook = get_axon_ntff_profile_hook
        sys.modules["antenv.axon_hooks"] = mod
        antenv.axon_hooks = mod
    except Exception:
        pass


_ensure_axon_hooks()

import numpy as np  # noqa: E402

NCH = 128           # channels
W = H = 64          # spatial
S = W * H           # 4096 spatial positions
B = 2               # batch
G = 4               # cores per sample
NCORES = 8
LS = S // G         # kernels per core (1024)
LT = LS // 128      # l-tiles per core (8)
ROWS = 8            # patch-center rows per chunk
CS = ROWS * H       # spatial chunk (512)
NCHUNK = W // ROWS  # 8 chunks
EPS = 1e-7

ROWB = 132          # spaced-row pitch in bytes (66 fp8 slots * 2)
PL = 2 + 66 * ROWB + 2   # spaced plane size (lead pad + 66 rows + tail)
NSTR = 263          # stream length per half-window (3*66 + 65)

_CACHE = {}
LAST_EXEC_TIME_NS = None

# d-pair table for the score GEMM: shift pairs (d = 3*dy + dx) and the byte
# delta between the two windows of each pair in the spaced layout
D_PAIRS = [(0, 1), (2, 3), (4, 5), (6, 7)]
D_DELTA = [2, 128, 2, 2]


def _build():
    from concourse import bacc, bass_isa, tile, mybir
    from concourse.ap import AP as RawAP
    from concourse.masks import make_identity

    F32 = mybir.dt.float32
    BF = mybir.dt.bfloat16
    F8 = mybir.dt.float8e4
    DRow = mybir.MatmulPerfMode.DoubleRow
    Alu = mybir.AluOpType
    Act = mybir.ActivationFunctionType
    AxX = mybir.AxisListType.X
    RMax = bass_isa.ReduceOp.max
    RAdd = bass_isa.ReduceOp.add

    nc = bacc.Bacc("TRN2", target_bir_lowering=False, debug=False,
                   num_devices=NCORES)

    fg_ext = nc.dram_tensor("fg", [NCH, S], F32, kind="ExternalInput")
    fgband_ext = nc.dram_tensor("fgband", [NCH, 18 * H], F32,
                                kind="ExternalInput")
    mask_ext = nc.dram_tensor("mask", [1, S], F32, kind="ExternalInput")
    mband_ext = nc.dram_tensor("maskband", [1, 18 * H], F32,
                               kind="ExternalInput")
    out_ext = nc.dram_tensor("out", [NCH // G, S], F32, kind="ExternalOutput")

    groups = [[0, 1, 2, 3], [4, 5, 6, 7]]

    def rap(base, off, dims):
        """Raw strided view of a [128, N] tile at element offset `off`."""
        return RawAP(base.tensor, base.offset + off,
                     [list(base.ap[0])] + [list(d) for d in dims])

    with tile.TileContext(nc) as tc:
        with tc.tile_pool(name="const", bufs=1) as cpool, \
             tc.tile_pool(name="pers", bufs=1) as pers, \
             tc.tile_pool(name="psA", bufs=2, space="PSUM") as psA, \
             tc.tile_pool(name="psB", bufs=4, space="PSUM") as psB, \
             tc.tile_pool(name="psS", bufs=2, space="PSUM") as psS, \
             tc.tile_pool(name="dram", bufs=2, space="DRAM") as dram, \
             tc.tile_pool(name="dramP", bufs=1, space="DRAM") as dramP:

            ident_b = cpool.tile([128, 128], BF, tag="idb")
            make_identity(nc, ident_b[:])
            ones_s = cpool.tile([128, 1], F8, tag="ones")
            nc.gpsimd.memset(ones_s[:], 1.0)

            # ---------------- persistent tensors ----------------
            box_sp = pers.tile([NCH, PL], F8, tag="boxsp")
            kern8 = pers.tile([NCH, 9, LS], F8, tag="kern8")
            kern_lc = pers.tile([128, 9, LT, NCH], F8, tag="kernlc")
            ahalo = pers.tile([128, LT * PL], F8, tag="ahalo")
            q32 = pers.tile([32, NCHUNK, CS // 32], F32, tag="q32")
            rnorm_col = pers.tile([128, LT], F32, tag="rnorm")

            box_b = box_sp[:]
            ah_b = ahalo[:]

            bar_in = dramP.tile([16], F32, tag="bari")
            bar_out = dramP.tile([16 * NCORES], F32, tag="baro")
            bar2_in = dramP.tile([4 * CS], F32, tag="bari2")
            bar2_out = dramP.tile([4 * CS * G], F32, tag="baro2")
            nrm_dram = dramP.tile([LS], F32, tag="nrmd")

            ctxK = tc.tile_pool(name="prepk", bufs=1)
            prepk = ctxK.__enter__()
            kernT = prepk.tile([NCH, 9, LS], BF, tag="kernT")
            with tc.tile_pool(name="prep", bufs=1) as prep:
                # ---- input loads (fg split across two queues) ----
                mband_row = prep.tile([1, 18 * H], F32, tag="mbandrow")
                nc.sync.dma_start(mband_row[:], mband_ext[:])
                fgband_sb = prep.tile([NCH, 18, H], F32, tag="fgband")
                nc.sync.dma_start(
                    fgband_sb[:],
                    fgband_ext[:].rearrange("c (r x) -> c r x", r=18))
                fg_sb = prep.tile([NCH, W, H], F32, tag="fgsb")
                fg3 = fg_ext[:].rearrange("c (y x) -> c y x", y=W)
                nc.scalar.dma_start(fg_sb[:, 0:32, :], fg3[:, 0:32, :])
                nc.sync.dma_start(fg_sb[:, 32:64, :], fg3[:, 32:64, :])
                m32a = prep.tile([32, NCHUNK, CS // 32], F32, tag="m32a")
                for k in range(NCHUNK):
                    nc.sync.dma_start(m32a[:, k, :],
                                      mask_ext[:, k * CS:(k + 1) * CS])

                # ---- kernels: kernT[c, d, l] = (band*mask)[shifted] + EPS --
                mband_bc = prep.tile([NCH, 18 * H], BF, tag="mbandbc")
                mband_bf = prep.tile([1, 18 * H], BF, tag="mbandbf")
                nc.scalar.activation(mband_bf[:], mband_row[:], Act.Identity)
                nc.gpsimd.partition_broadcast(mband_bc[:], mband_bf[:])
                # fp8 feature staging issued early: the scalar-engine fgbp
                # convert gates the whole vector box-filter chain
                fgbp = prep.tile([NCH, W, 68], BF, tag="fgbp")
                nc.gpsimd.memset(fgbp[:], 0.0)
                nc.scalar.activation(fgbp[:, :, 2:66], fg_sb[:], Act.Identity)
                bgbandp = prep.tile([NCH, 18, 66], F32, tag="bgbandp")
                nc.gpsimd.memset(bgbandp[:], 0.0)
                nc.vector.tensor_mul(
                    bgbandp[:, :, 1:65], fgband_sb[:],
                    mband_bc[:].rearrange("c (r x) -> c r x", r=18))
                for d in range(9):
                    dy, dx = d // 3, d % 3
                    nc.vector.tensor_scalar_add(
                        kernT[:, d, :],
                        bgbandp[:, dy:dy + 16, dx:dx + 64], EPS)
                for d in range(9):
                    nc.scalar.activation(kern8[:, d, :], kernT[:, d, :],
                                         Act.Identity)

                # ---- warmup collectives (after the critical gpsimd ops) --
                nc.gpsimd.dma_start(bar_in[:], mband_row[0:1, 0:16])
                nc.gpsimd.collective_compute(
                    "AllGather", Alu.bypass,
                    replica_groups=[list(range(NCORES))],
                    ins=[bar_in.opt()], outs=[bar_out.opt()])
                junk = prep.tile([32, 64], F32, tag="junk")
                nc.gpsimd.memset(junk[:], 0.0)
                nc.gpsimd.dma_start(bar2_in[:], junk[:])
                nc.gpsimd.collective_compute(
                    "AllGather", Alu.bypass, replica_groups=groups,
                    ins=[bar2_in.opt()], outs=[bar2_out.opt()])

                # ---- kernel norms: sumsq via squares + ones-matmul ----
                onesb = prep.tile([128, 1], BF, tag="onesb")
                nc.gpsimd.memset(onesb[:], 1.0)
                ksq = prep.tile([NCH, LS], BF, tag="ksq")
                ps_s0 = psS.tile([1, 512], F32, tag="psS")
                ps_s1 = psS.tile([1, 512], F32, tag="psS")
                for d in range(9):
                    nc.vector.tensor_mul(ksq[:], kernT[:, d, :],
                                         kernT[:, d, :])
                    nc.tensor.matmul(ps_s0[:], onesb[:], ksq[:, 0:512],
                                     start=(d == 0), stop=(d == 8))
                    nc.tensor.matmul(ps_s1[:], onesb[:], ksq[:, 512:1024],
                                     start=(d == 0), stop=(d == 8))
                srow = prep.tile([1, LS], F32, tag="srow")
                nc.scalar.activation(srow[:, 0:512], ps_s0[:], Act.Identity)
                nc.scalar.activation(srow[:, 512:1024], ps_s1[:],
                                     Act.Identity)
                nc.gpsimd.dma_start(nrm_dram[:], srow[:])
                sq128 = prep.tile([128, LT], F32, tag="sq128")
                nc.gpsimd.dma_start(
                    sq128[:],
                    nrm_dram[:].rearrange("(t p) -> p t", p=128))
                norm128 = prep.tile([128, LT], F32, tag="norm128")
                nc.scalar.activation(norm128[:], sq128[:], Act.Sqrt)

                # ---- box filter via zero-padded shifts, then spaced fp8 --
                t1 = prep.tile([NCH, W, 66], BF, tag="t1")
                nc.vector.tensor_add(t1[:], fgbp[:, :, 0:66],
                                     fgbp[:, :, 1:67])
                tmpHp = prep.tile([NCH, 68, 66], BF, tag="tmpHp")
                nc.gpsimd.memset(tmpHp[:], 0.0)
                nc.vector.tensor_add(tmpHp[:, 2:66, :], t1[:],
                                     fgbp[:, :, 2:68])
                t2 = prep.tile([NCH, 66, 66], BF, tag="t1")
                nc.vector.tensor_add(t2[:], tmpHp[:, 0:66, :],
                                     tmpHp[:, 1:67, :])
                nc.gpsimd.memset(box_sp[:, 0:2], 0.0)
                # final V-pass add writes the spaced fp8 layout directly
                nc.vector.tensor_add(
                    rap(box_b, 2, [[ROWB, 66], [2, 66]]), t2[:],
                    tmpHp[:, 2:68, :])
                # reciprocal last on the vector queue so the box filter is
                # not blocked behind the norm-flatten DMA latency
                nc.vector.reciprocal(rnorm_col[:], norm128[:])

                # ---- blend constant: q32 = (1-mask)/9 in stat layout ----
                nc.vector.tensor_scalar(q32[:], m32a[:], -1.0 / 9.0,
                                        1.0 / 9.0, op0=Alu.mult, op1=Alu.add)

                # ---- attn-map border zeros (interior is overwritten) ----
                # row y=-1 and y=64 planes, plus the wrap slots (x'=-1 of
                # every row == x'=64 of the row above) and the lead pad
                nc.gpsimd.memset(rap(ah_b, 2, [[PL, LT], [1, ROWB]]), 0.0)
                nc.gpsimd.memset(
                    rap(ah_b, 2 + 65 * ROWB, [[PL, LT], [1, ROWB]]), 0.0)
                # bytes h*132..h*132+3 = x'=64 slot of row h-1 (+0) and
                # x'=-1 slot of row h (+2), for every row incl. lead pad
                nc.gpsimd.memset(
                    rap(ah_b, 0, [[PL, LT], [ROWB, 67], [1, 4]]), 0.0)

            # ---------------- chunk-loop pools ----------------
            ctx_scs = tc.tile_pool(name="scs", bufs=2)
            scsp = ctx_scs.__enter__()
            ctx_st = tc.tile_pool(name="stat", bufs=2)
            st = ctx_st.__enter__()
            ctx_bl = tc.tile_pool(name="blend", bufs=2)
            bl = ctx_bl.__enter__()

            def ps_win(ps, h):
                """[128, 4, 64] view of the half-window outputs in psum."""
                return ps[:, 1:265].rearrange(
                    "p (r x) -> p r x", x=66)[:, :, 0:64]

            def emit_gemm1(k):
                """scores for chunk k -> scs (normalized) + mtmp (max)."""
                r0 = k * ROWS
                scs = scsp.tile([128, LT, CS], F32, tag="scs")
                mtmp = st.tile([128, CS], F32, tag="mtmp")
                for t in range(LT):
                    ts = slice(t * 128, (t + 1) * 128)
                    for h in range(2):
                        ps = psA.tile([128, 512], F32, tag="psA")
                        for i, (d0, d1) in enumerate(D_PAIRS):
                            dy, dx = d0 // 3, d0 % 3
                            o = (r0 + 4 * h + dy) * ROWB + 2 * dx
                            nc.tensor.matmul(
                                ps[:, 0:NSTR], kern8[:, d0:d0 + 2, ts],
                                rap(box_b, o,
                                    [[D_DELTA[i], 2], [2, NSTR]]),
                                start=(i == 0), stop=False, perf_mode=DRow)
                        o8 = (r0 + 4 * h + 2) * ROWB + 4
                        nc.tensor.matmul(
                            ps[:, 0:NSTR], kern8[:, 8, ts],
                            box_sp[:, o8:o8 + 2 * NSTR:2],
                            start=False, stop=True)
                        nc.scalar.activation(
                            scs[:, t, :].rearrange(
                                "p (r x) -> p r x", r=ROWS)[:, 4 * h:4 * h + 4, :],
                            ps_win(ps, h), Act.Identity)
                    if t == 0:
                        nc.vector.tensor_scalar_mul(mtmp[:], scs[:, 0, :],
                                                    rnorm_col[:, 0:1])
                    else:
                        nc.vector.scalar_tensor_tensor(
                            mtmp[:], scs[:, t, :], rnorm_col[:, t:t + 1],
                            mtmp[:], op0=Alu.mult, op1=Alu.max)
                return scs, mtmp

            def emit_maxpath(k, mtmp):
                m_bc = st.tile([128, CS], F32, tag="mbc")
                nc.gpsimd.partition_all_reduce(m_bc[:], mtmp[:], 128, RMax)
                return m_bc

            def ah_int(k, t):
                """interior attn window of chunk k, tile t (strided)."""
                r0 = k * ROWS
                return rap(ah_b, t * PL + (1 + r0) * ROWB + 4,
                           [[ROWB, ROWS], [2, 64]])

            def emit_subexp(k, scs, m_bc):
                for t in range(LT):
                    diff = st.tile([128, CS], F32, tag="diff")
                    nc.vector.scalar_tensor_tensor(
                        diff[:], scs[:, t, :], rnorm_col[:, t:t + 1],
                        m_bc[:], op0=Alu.mult, op1=Alu.subtract)
                    nc.scalar.activation(
                        ah_int(k, t),
                        diff[:].rearrange("p (r x) -> p r x", r=ROWS),
                        Act.Exp)

            def emit_sums(k, ag_in, m_bc):
                slot = k % 2
                ps_sum = psS.tile([1, CS], F32, tag="psS")
                for t in range(LT):
                    nc.tensor.matmul(
                        ps_sum[:], ones_s[:], ah_int(k, t),
                        start=(t == 0), stop=(t == LT - 1))
                s_row = st.tile([1, CS], F32, tag="srowc")
                nc.scalar.activation(s_row[:], ps_sum[:], Act.Identity)
                nc.gpsimd.dma_start(
                    ag_in[slot * 2 * CS:slot * 2 * CS + CS], m_bc[0:1, :])
                nc.gpsimd.dma_start(
                    ag_in[slot * 2 * CS + CS:(slot + 1) * 2 * CS], s_row[:])

            def emit_ag(pair):
                ag_out = dram.tile([4 * CS * G], F32, tag="ago")
                nc.gpsimd.collective_compute(
                    "AllGather", Alu.bypass, replica_groups=groups,
                    ins=[pair["in"].opt()], outs=[ag_out.opt()])
                pair["out"] = ag_out

            def emit_combine(k, pair):
                """gathered stats -> w_bc = fac * (1-mask)/9 broadcast.
                [32, 16] stat layout (linear col order, like the dumps)."""
                slot = k % 2
                ag_in, ag_out = pair["in"], pair["out"]
                cm = st.tile([32, G, CS // 32], F32, tag="cm")
                cs = st.tile([32, G, CS // 32], F32, tag="cs")
                for r in range(G):
                    base = r * 4 * CS + slot * 2 * CS
                    nc.gpsimd.dma_start(cm[:, r, :], ag_out[base:base + CS])
                    nc.gpsimd.dma_start(cs[:, r, :],
                                        ag_out[base + CS:base + 2 * CS])
                m32 = st.tile([32, CS // 32], F32, tag="m32")
                nc.gpsimd.dma_start(
                    m32[:], ag_in[slot * 2 * CS:slot * 2 * CS + CS])
                Mx = st.tile([32, CS // 32], F32, tag="Mx")
                nc.vector.tensor_reduce(
                    Mx[:], cm[:].rearrange("p r t -> p t r"), AxX, Alu.max)
                for r in range(G):
                    nc.vector.tensor_sub(cm[:, r, :], cm[:, r, :], Mx[:])
                nc.scalar.activation(cm[:], cm[:], Act.Exp)
                nc.vector.tensor_mul(cs[:], cs[:], cm[:])
                gs = st.tile([32, CS // 32], F32, tag="gs")
                nc.vector.tensor_reduce(
                    gs[:], cs[:].rearrange("p r t -> p t r"), AxX, Alu.add)
                rg = st.tile([32, CS // 32], F32, tag="rg")
                nc.vector.reciprocal(rg[:], gs[:])
                w_sl = st.tile([32, CS // 32], F32, tag="wsl")
                nc.vector.tensor_sub(w_sl[:], m32[:], Mx[:])
                nc.scalar.activation(w_sl[:], w_sl[:], Act.Exp)
                nc.vector.tensor_mul(w_sl[:], w_sl[:], rg[:])
                nc.vector.tensor_mul(w_sl[:], w_sl[:], q32[:, k, :])
                w_dram = dram.tile([CS], F32, tag="wd")
                nc.gpsimd.dma_start(w_dram[:], w_sl[:])
                w_row = st.tile([1, CS], F32, tag="wrow")
                nc.gpsimd.dma_start(w_row[:], w_dram[:])
                w_bc = st.tile([128, CS], F32, tag="wbc")
                nc.gpsimd.partition_broadcast(w_bc[:], w_row[:])
                return w_bc

            def emit_blend_prefetch(k):
                mrowk = bl.tile([1, CS], F32, tag="mrowk")
                nc.sync.dma_start(mrowk[:], mask_ext[:, k * CS:(k + 1) * CS])
                mbc = bl.tile([128, CS], F32, tag="mbck")
                nc.gpsimd.partition_broadcast(mbc[:], mrowk[:])
                fgc = bl.tile([NCH, CS], F32, tag="fgc")
                nc.sync.dma_start(fgc[:], fg_ext[:, k * CS:(k + 1) * CS])
                return mbc, fgc

            def emit_gemm2(k):
                r0 = k * ROWS
                g2sb = bl.tile([128, CS], F32, tag="g2sb")
                gv = g2sb[:].rearrange("p (r x) -> p r x", r=ROWS)
                for h in range(2):
                    ps2 = psB.tile([128, 512], F32, tag="psB")
                    n = 0
                    for dyp in range(3):
                        for dxp in range(3):
                            dflip = (2 - dyp) * 3 + (2 - dxp)
                            o = (r0 + 4 * h + dyp) * ROWB + 2 * dxp
                            for tp in range(LT // 2):
                                nc.tensor.matmul(
                                    ps2[:, 0:NSTR],
                                    kern_lc[:, dflip, 2 * tp:2 * tp + 2, :],
                                    rap(ah_b, 2 * tp * PL + o,
                                        [[PL, 2], [2, NSTR]]),
                                    start=(n == 0), stop=(n == 35),
                                    perf_mode=DRow)
                                n += 1
                    # drain psum immediately so the pool never stalls the PE
                    nc.vector.tensor_copy(gv[:, 4 * h:4 * h + 4, :],
                                          ps_win(ps2, h))
                return g2sb

            def emit_blend(k, g2sb, w_bc, mbc, fgc, rs_in, slot):
                mfk = bl.tile([128, CS], F32, tag="mfk")
                nc.vector.scalar_tensor_tensor(
                    mfk[:], fgc[:], 1.0 / G, mbc[:], op0=Alu.mult,
                    op1=Alu.mult)
                out_sb = bl.tile([128, CS], F32, tag="outsb")
                nc.vector.tensor_mul(out_sb[:], g2sb[:], w_bc[:])
                nc.vector.tensor_add(out_sb[:], out_sb[:], mfk[:])
                nc.sync.dma_start(
                    rs_in[:, slot * CS:(slot + 1) * CS], out_sb[:])

            pending_out = []

            def emit_rs(j0, nch, rs_in):
                rs_out = dram.tile([NCH // G, nch * CS], F32, tag="rso")
                nc.gpsimd.collective_compute(
                    "ReduceScatter", Alu.add, replica_groups=groups,
                    ins=[rs_in.opt()], outs=[rs_out.opt()])
                pending_out.append((j0, nch, rs_out))

            def flush_out(keep):
                while len(pending_out) > keep:
                    j0, nch, rs_out = pending_out.pop(0)
                    nc.sync.dma_start(
                        out_ext[:, j0 * CS:(j0 + nch) * CS], rs_out[:])

            # ---------------- software pipeline ----------------
            g1 = {0: emit_gemm1(0)}
            # kern_lc[l, d, c] via PE transposes (after g1(0) on the PE
            # queue; the rnorm-scaled copies are off every critical path)
            for d in range(9):
                for t in range(LT):
                    pt = psA.tile([128, 256], BF, tag="psA")
                    nc.tensor.transpose(
                        pt[:, 0:128],
                        kernT[:, d, t * 128:(t + 1) * 128], ident_b[:])
                    nc.scalar.activation(
                        kern_lc[:, d, t, :], pt[:, 0:128], Act.Identity,
                        scale=rnorm_col[:, t:t + 1])
            mp = {0: emit_maxpath(0, g1[0][1])}
            ag_pairs = {}
            rs_bufs = {}
            ps2s = {}
            blf = {}
            for k in range(NCHUNK + 2):
                if k <= NCHUNK - 1:
                    if k % 2 == 0:
                        ag_pairs[k // 2] = {
                            "in": dram.tile([4 * CS], F32, tag="agi",
                                            name=f"agi{k // 2}")}
                        rs_bufs[k // 2] = dram.tile(
                            [NCH, 2 * CS], F32, tag="rsi",
                            name=f"rsi{k // 2}")
                    emit_subexp(k, g1[k][0], mp[k])
                # combine/blend for chunk k-3 first: all its inputs (the
                # pair AllGather, the drained gemm2 output) are iterations
                # old, so the whole chain runs at iteration start
                def do_blend(j):
                    w_bc = emit_combine(j, ag_pairs[j // 2])
                    mbc, fgc = blf.pop(j)
                    emit_blend(j, ps2s.pop(j), w_bc, mbc, fgc,
                               rs_bufs[j // 2], j % 2)
                    if j % 2 == 1:
                        emit_rs(j - 1, 2, rs_bufs.pop(j // 2))

                if 0 <= k - 3 <= NCHUNK - 1:
                    do_blend(k - 3)
                flush_out(1)
                if 0 <= k - 2 <= NCHUNK - 1:
                    blf[k - 2] = emit_blend_prefetch(k - 2)
                    ps2s[k - 2] = emit_gemm2(k - 2)
                if k == NCHUNK + 1:
                    # drain: last chunk blends right after its gemm2
                    do_blend(NCHUNK - 1)
                    flush_out(0)
                if k <= NCHUNK - 1:
                    emit_sums(k, ag_pairs[k // 2]["in"], mp.pop(k))
                    g1.pop(k)
                    if k % 2 == 1:
                        emit_ag(ag_pairs[k // 2])
                if k + 1 <= NCHUNK - 1:
                    g1[k + 1] = emit_gemm1(k + 1)
                    mp[k + 1] = emit_maxpath(k + 1, g1[k + 1][1])

            flush_out(0)

            ctx_bl.__exit__(None, None, None)
            ctx_st.__exit__(None, None, None)
            ctx_scs.__exit__(None, None, None)
            ctxK.__exit__(None, None, None)

    nc.compile()
    return nc


def _shard_inputs(fg, mk):
    """fg [2,128,64,64] f32, mk [2,1,64,64] f32 -> per-core input maps."""
    in_maps = []
    for core in range(NCORES):
        b, r = core // G, core % G
        y0 = r * (W // G)
        feat = np.ascontiguousarray(fg[b].reshape(NCH, S), np.float32)
        mask = np.ascontiguousarray(mk[b].reshape(1, S), np.float32)
        band = np.zeros((NCH, 18, H), np.float32)
        mband = np.zeros((1, 18, H), np.float32)
        lo = y0 - 1
        src_lo = max(0, lo)
        src_hi = min(W, y0 + 17)
        band[:, src_lo - lo:src_hi - lo] = fg[b][:, src_lo:src_hi]
        mband[:, src_lo - lo:src_hi - lo] = mk[b][:, src_lo:src_hi]
        in_maps.append({
            "fg": feat,
            "fgband": np.ascontiguousarray(band.reshape(NCH, 18 * H)),
            "mask": mask,
            "maskband": np.ascontiguousarray(mband.reshape(1, 18 * H)),
        })
    return in_maps


def kernel(foreground, masks):
    global LAST_EXEC_TIME_NS
    from concourse.bass_utils import run_bass_kernel_spmd

    fg = np.asarray(foreground, np.float32)
    mk = np.asarray(masks, np.float32)
    assert fg.shape == (B, NCH, W, H) and mk.shape == (B, 1, W, H)

    nc = _CACHE.get("nc")
    if nc is None:
        nc = _build()
        _CACHE["nc"] = nc

    in_maps = _shard_inputs(fg, mk)
    trace = bool(os.environ.get("BASS_KERNEL_TRACE"))
    res = run_bass_kernel_spmd(nc, in_maps, core_ids=list(range(NCORES)),
                               trace=trace)
    LAST_EXEC_TIME_NS = res.exec_time_ns
    if res.exec_time_ns is not None:
        print(f"HW exec time: {res.exec_time_ns} ns")

    out = np.empty((B, NCH, W, H), np.float32)
    for core in range(NCORES):
        b, r = core // G, core % G
        out[b, 32 * r:32 * (r + 1)] = (
            res.results[core]["out"].reshape(32, W, H))
    return out
